# revision 31
# baseline (speedup 1.0000x reference)
"""Trainium2 Bass kernel for nn_DeepManualLSTM (3-layer LSTM, B=1024, T=48, IN=64, H=512).

Strategy: data-parallel over batch (128 rows/core x 8 cores), weights SBUF
resident.  Mixed-precision gate GEMM: the f/i/o gate banks run as fp8-e4m3
DoubleRow matmuls (K=256 per pass, ~2.3x the fp32r column rate on HW) with
h and W scaled by 16 (descale folded into the sigmoid's input scale); the
c gate (tanh -> additive into C, the only error-sensitive path) stays bf16
at full rate.  Orientation: activations transposed (feature-major) as the
stationary operand, weights moving with N<=512 per PSUM bank.  The f/i/o
banks share one [128,3,512] PSUM tile so a single chunked sigmoid covers
all three.  h is produced batch-major, then PE-transposed (bf16 + scaled
fp8 copies) two cells later in the wavefront so the ACT/DVE tail latency
hides under other cells' matmuls.  The 48-step recurrence is fully
unrolled; the final [B,H]@[H,1] projection runs on the host.
"""
import sys
import os

for _p in ("/opt/trn_rl_repo", "/root/.axon_site/_ro/trn_rl_repo"):
    if os.path.isdir(_p) and _p not in sys.path:
        sys.path.insert(0, _p)

import numpy as np
import ml_dtypes

import concourse.bass as bass
import concourse.tile as tile
from concourse import bacc, mybir
from concourse import bass_utils
from concourse.bass import ds, ts
from concourse.masks import make_identity

P = 128          # batch rows per core / SBUF partitions
T = 48           # sequence length
IN = 64          # input features
H = 512          # hidden size
L = 3            # layers
G4 = 4 * H       # gate width (2048)
NB = 4           # PSUM banks per gate row (G4 / 512)
KH = H // P      # k-chunks of the hidden contraction (4)
KP = KH // 2     # k-pairs for DoubleRow (2)
NCORES = 8
SH = 16.0        # fp8 scale on h
SW = 16.0        # fp8 scale on W
SG = SH * SW     # fio gate descale

F32 = mybir.dt.float32
F32R = mybir.dt.float32r
BF16 = mybir.dt.bfloat16
F8 = mybir.dt.float8e4
DR = mybir.MatmulPerfMode.DoubleRow
AF = mybir.ActivationFunctionType
E4M3 = ml_dtypes.float8_e4m3
NPBF16 = ml_dtypes.bfloat16

# fio global columns -> source columns in the [*, 2048] weight matrices
FIO_COLS = (slice(0, 512), slice(512, 1024), slice(1536, 2048))
C_COLS = slice(1024, 1536)


def _round_fp32r(a: np.ndarray) -> np.ndarray:
    u = np.ascontiguousarray(a, dtype=np.float32).view(np.uint32)
    u = u + 0x7FF + ((u >> 12) & 1)
    u &= np.uint32(0xFFFFF000)
    return u.view(np.float32)


def _build(include_bias: bool, reps: int = 1) -> bass.Bass:
    nc = bacc.Bacc()

    # x pre-transposed on the host: [128, (T//2)*128] fp32r, even t in
    # partitions 0:64, odd t in 64:128, column block t//2 holds x_t^T.
    xT_d = nc.dram_tensor("xT", [P, (T // 2) * P], F32R, kind="ExternalInput")
    # Wx0 with f/i/o columns pre-scaled by SG (c columns x1), fp32r.
    wx0_d = nc.dram_tensor("wx0", [IN, G4], F32R, kind="ExternalInput")
    # per-layer packed weights: c bank bf16 [P, KH*512]; fio fp8 [P, KP*2*1536]
    wc_d, wf_d = {}, {}
    for name in ("wh0", "wx1", "wh1", "wx2", "wh2"):
        wc_d[name] = nc.dram_tensor(f"{name}c", [P, KH * 512], BF16, kind="ExternalInput")
        wf_d[name] = nc.dram_tensor(f"{name}f", [P, KP * 2 * 1536], F8, kind="ExternalInput")
    if include_bias:
        bf_d = [nc.dram_tensor(f"bf{l}", [1, 1536], F32R, kind="ExternalInput") for l in range(L)]
        bc_d = [nc.dram_tensor(f"bc{l}", [1, 512], F32R, kind="ExternalInput") for l in range(L)]
    out_d = nc.dram_tensor("hout", [P, H], BF16, kind="ExternalOutput")

    with tile.TileContext(nc) as tc:
        with (
            tc.tile_pool(name="wpool", bufs=1) as wp,
            tc.tile_pool(name="state", bufs=1) as st,
            tc.tile_pool(name="work", bufs=1) as wk,
            tc.tile_pool(name="psg", bufs=2, space="PSUM") as psg,
        ):
            # ---- persistent tiles ------------------------------------------------
            ident = wp.tile([P, P], F32)
            make_identity(nc, ident)
            ident16 = wp.tile([P, P], BF16)
            nc.scalar.copy(ident16[:], ident[:])
            ident8 = wp.tile([P, P], F8)
            nc.scalar.copy(ident8[:], ident[:])

            xT_t = wp.tile([P, (T // 2) * P], F32R)
            nc.sync.dma_start(xT_t[:], xT_d[:])

            # Wx0 duplicated into both partition halves so odd-t x tiles
            # (living at base partition 64) find it on matching partitions.
            wx0_t = wp.tile([P, G4], F32R)
            nc.sync.dma_start(wx0_t[:IN, :], wx0_d[:])
            nc.sync.dma_start(wx0_t[IN:, :], wx0_d[:])

            wc_t, wf_t = {}, {}
            for name in ("wh0", "wx1", "wh1", "wx2", "wh2"):
                ct = wp.tile([P, KH, 512], BF16, name=f"{name}c_t")
                nc.sync.dma_start(ct[:], wc_d[name][:])
                wc_t[name] = ct
                ft = wp.tile([P, KP, 6, 2, 256], F8, name=f"{name}f_t")
                nc.sync.dma_start(ft[:], wf_d[name][:])
                wf_t[name] = ft

            if include_bias:
                ones_f = wp.tile([1, P], F32)
                nc.vector.memset(ones_f[:], 1.0)
                ones_t = wp.tile([1, P], F32R)
                nc.scalar.copy(ones_t[:], ones_f[:])
                bf_t, bc_t = [], []
                for l in range(L):
                    t1 = wp.tile([1, 1536], F32R, name=f"bf{l}_t")
                    nc.sync.dma_start(t1[:], bf_d[l][:])
                    bf_t.append(t1)
                    t2 = wp.tile([1, 512], F32R, name=f"bc{l}_t")
                    nc.sync.dma_start(t2[:], bc_d[l][:])
                    bc_t.append(t2)

            # states: hT bf16 and hT8 fp8 (x16), both [P, KH, P]; C fp32.
            zbuf = wk.tile([P, H], F32, tag="zb")
            nc.vector.memset(zbuf[:], 0.0)
            hT, hT8, Cs = [], [], []
            for l in range(L):
                h_t = st.tile([P, KH, P], BF16, name=f"hT{l}")
                nc.scalar.copy(h_t[:], zbuf[:])
                hT.append(h_t)
                h8_t = st.tile([P, KH, P], F8, name=f"hT8{l}")
                nc.scalar.copy(h8_t[:], zbuf[:])
                hT8.append(h8_t)
                c_t = st.tile([P, H], F32, name=f"C{l}")
                nc.vector.memset(c_t[:], 0.0)
                Cs.append(c_t)

            pending = []  # up to 2 deferred (layer, finish) entries
            last_gc = [None]  # gc tile of the previously emitted cell

            def flush_one():
                # park the transposes in the previous cell's gc bank: its
                # tanh_c read completes early, and the next user of that
                # PSUM slot (2 cells on) only writes it ~1us into its cell.
                pending.pop(0)[1](last_gc[0])

            def cell(t: int, l: int):
                gf = psg.tile([P, 3, 512], F32, name="gfio", tag="gfio")
                gc = psg.tile([P, 512], F32, name="gc", tag="gc")

                # ---- input GEMM (fio banks first, c last) -------------------
                if include_bias:
                    for b in range(3):
                        nc.tensor.matmul(
                            gf[:, b, :], ones_t[:], bf_t[l][:, ts(b, 512)],
                            start=True, stop=False,
                        )
                    nc.tensor.matmul(
                        gc[:], ones_t[:], bc_t[l][:], start=True, stop=False
                    )
                if l == 0:
                    r0 = 0 if t % 2 == 0 else 64
                    xc = xT_t[r0 : r0 + IN, ts(t // 2, P)]
                    for b, cols in enumerate(FIO_COLS):
                        nc.tensor.matmul(
                            gf[:, b, :], xc, wx0_t[r0 : r0 + IN, cols],
                            start=not include_bias, stop=False,
                            skip_group_check=True,
                        )
                    nc.tensor.matmul(
                        gc[:], xc, wx0_t[r0 : r0 + IN, C_COLS],
                        start=not include_bias, stop=False,
                        skip_group_check=True,
                    )
                else:
                    wf = wf_t[f"wx{l}"]
                    for j in range(KP):
                        lhs8 = hT8[l - 1][:, 2 * j : 2 * j + 2, :]
                        for q in range(6):
                            b, off = q // 2, (q % 2) * 256
                            # one start per PSUM bank (zero region =
                            # 2KB): the off=256 chunk's first write lands
                            # on pending-zero bytes and self-zeroes.
                            nc.tensor.matmul(
                                gf[:, b, off : off + 256],
                                lhs8,
                                wf[:, j, q, :, :],
                                start=(j == 0 and off == 0 and not include_bias),
                                stop=False,
                                skip_group_check=True,
                                perf_mode=DR,
                            )
                    wc = wc_t[f"wx{l}"]
                    for j in range(KH):
                        nc.tensor.matmul(
                            gc[:], hT[l - 1][:, j, :], wc[:, j, :],
                            start=(j == 0 and not include_bias), stop=False,
                            skip_group_check=True,
                        )

                # Deferred finishes land here — between this cell's input and
                # state GEMMs — so a finish's h_b (produced ~2 cells back) is
                # ready, its hT copies overlap this cell's state matmuls, and
                # any pending write to hT[l] precedes this cell's state reads.
                while pending and (
                    len(pending) >= 1 or any(pl == l for pl, _ in pending)
                ):
                    flush_one()

                # ---- state GEMM (c first so tanh_c starts early) ------------
                wc = wc_t[f"wh{l}"]
                for j in range(KH):
                    nc.tensor.matmul(
                        gc[:], hT[l][:, j, :], wc[:, j, :],
                        start=False, stop=(j == KH - 1), skip_group_check=True,
                    )
                wf = wf_t[f"wh{l}"]
                for j in range(KP):
                    lhs8 = hT8[l][:, 2 * j : 2 * j + 2, :]
                    for q in range(6):
                        b, off = q // 2, (q % 2) * 256
                        nc.tensor.matmul(
                            gf[:, b, off : off + 256],
                            lhs8,
                            wf[:, j, q, :, :],
                            start=False, stop=(j == KP - 1),
                            skip_group_check=True,
                            perf_mode=DR,
                        )

                # ---- gate tail ----------------------------------------------
                fio_s = wk.tile([P, 3, H], F32, name="fio_s")
                c_s = wk.tile([P, H], F32, name="c_s")
                ic = wk.tile([P, H], F32, name="ic")
                fC = wk.tile([P, H], F32, name="fC")
                tanC = wk.tile([P, H], F32, name="tanC")
                h_b = wk.tile([P, H], BF16, name="h_b", bufs=2)
                h8 = wk.tile([P, H], F8, name="h8", bufs=2)

                nc.scalar.activation(c_s[:], gc[:], AF.Tanh)
                nc.scalar.activation(
                    fio_s[:], gf[:], AF.Sigmoid, scale=1.0 / SG
                )
                f_s, i_s, o_s = fio_s[:, 0, :], fio_s[:, 1, :], fio_s[:, 2, :]
                # elementwise cell update split DVE/Pool to balance engines
                for q in range(2):
                    qs = slice(q * 256, (q + 1) * 256)
                    nc.vector.tensor_mul(ic[:, qs], i_s[:, qs], c_s[:, qs])
                    nc.vector.tensor_mul(fC[:, qs], f_s[:, qs], Cs[l][:, qs])
                    nc.vector.tensor_add(Cs[l][:, qs], fC[:, qs], ic[:, qs])
                    nc.scalar.activation(tanC[:, qs], Cs[l][:, qs], AF.Tanh)
                    nc.vector.tensor_mul(h_b[:, qs], o_s[:, qs], tanC[:, qs])
                    nc.scalar.mul(h8[:, qs], h_b[:, qs], SH)

                def finish(tp, l=l, h_b=h_b, h8=h8):
                    tp_bf = tp[:, 0:256].bitcast(BF16)
                    # fp8 transpose outputs must have element step 2 (hw rule)
                    tp_f8 = tp[:, 256:512].bitcast(F8)
                    for j in range(KH):
                        nc.tensor.transpose(
                            tp_bf[:, ts(j, P)], h_b[:, ts(j, P)], ident16[:]
                        )
                    for j in range(KH):
                        nc.tensor.transpose(
                            tp_f8[:, j * 256 : (j + 1) * 256 : 2],
                            h8[:, ts(j, P)],
                            ident8[:],
                        )
                    nc.vector.tensor_copy(hT[l][:], tp_bf[:])
                    nc.vector.tensor_copy(hT8[l][:], tp_f8[:, 0:1024:2])

                pending.append((l, finish))
                last_gc[0] = gc

            def whole_pass():
                # wavefront order: cells (s,0), (s-1,1), (s-2,2)
                for s in range(T + L - 1):
                    for l in range(L):
                        t = s - l
                        if 0 <= t < T:
                            cell(t, l)
                while pending:
                    flush_one()

            if reps > 1:
                with tc.For_i(0, reps, 1):
                    whole_pass()
            else:
                whole_pass()

            # ---- ship final top-layer h back to the host -------------------
            nc.sync.dma_start(out_d[:], hT[L - 1][:])

    nc.finalize()
    return nc


_NC_CACHE: dict = {}
_LAST_RUN: dict = {}


def _pack_xT(x_shard: np.ndarray) -> np.ndarray:
    """[128, T, IN] -> [128, (T//2)*128] packed transposed layout (fp32r)."""
    xt = np.zeros((P, (T // 2) * P), dtype=np.float32)
    for t in range(T):
        r0 = 0 if t % 2 == 0 else 64
        xt[r0 : r0 + IN, (t // 2) * P : (t // 2 + 1) * P] = x_shard[:, t, :].T
    return _round_fp32r(xt)


def _pack_w(W: np.ndarray) -> tuple[np.ndarray, np.ndarray]:
    """[H, 2048] -> (c bf16 [P, KH*512], fio fp8 [P, KP*2*1536])."""
    W = np.asarray(W, np.float32)
    arr = W.reshape(KH, P, G4)
    wc = (
        arr[:, :, C_COLS].transpose(1, 0, 2).reshape(P, KH * 512).astype(NPBF16)
    )
    Wf = np.concatenate([W[:, c] for c in FIO_COLS], axis=1)  # [H, 1536]
    # [k, j, q, i, n] = 16*Wf[128*(2j+i)+k, 256q+n] — contiguous [2,256]
    # blocks so the DoubleRow moving fetch streams sequentially.
    wf = (
        (Wf.reshape(KP, 2, P, 6, 256) * SW)
        .transpose(2, 0, 3, 1, 4)
        .reshape(P, KP * 6 * 2 * 256)
        .astype(E4M3)
    )
    return wc, wf


def kernel(**inputs) -> np.ndarray:
    x = np.ascontiguousarray(np.asarray(inputs["x"], dtype=np.float32))
    B = x.shape[0]
    assert B % NCORES == 0
    Bl = B // NCORES

    wx0 = np.asarray(inputs["Wx0"], dtype=np.float32).copy()
    for c in FIO_COLS:
        wx0[:, c] *= SG
    wx0 = _round_fp32r(wx0)
    packed = {}
    for name, key in (("wh0", "Wh0"), ("wx1", "Wx1"), ("wh1", "Wh1"),
                      ("wx2", "Wx2"), ("wh2", "Wh2")):
        packed[name] = _pack_w(inputs[key])
    fc_w = np.asarray(inputs["fc_w"], dtype=np.float32)
    bs = [np.asarray(inputs[f"b{l}"], dtype=np.float32) for l in range(L)]
    fc_b = np.asarray(inputs["fc_b"], dtype=np.float32)
    include_bias = any(np.any(b != 0) for b in bs)

    key = include_bias
    if key not in _NC_CACHE:
        _NC_CACHE[key] = _build(include_bias)
    nc = _NC_CACHE[key]
    _LAST_RUN["include_bias"] = include_bias

    in_maps = []
    for c in range(NCORES):
        m = {"xT": _pack_xT(x[c * Bl : (c + 1) * Bl]), "wx0": wx0}
        for name, (wc, wf) in packed.items():
            m[f"{name}c"] = wc
            m[f"{name}f"] = wf
        if include_bias:
            for l in range(L):
                b = bs[l]
                bf = np.concatenate([b[c] for c in FIO_COLS]) * SG
                m[f"bf{l}"] = _round_fp32r(bf).reshape(1, 1536)
                m[f"bc{l}"] = _round_fp32r(b[C_COLS]).reshape(1, 512)
        in_maps.append(m)

    res = bass_utils.run_bass_kernel_spmd(nc, in_maps, core_ids=list(range(NCORES)))
    _LAST_RUN["nc"] = nc
    _LAST_RUN["in_maps"] = in_maps
    outs = []
    for c in range(NCORES):
        ht = np.asarray(res.results[c]["hout"]).astype(np.float32)
        # [128, 512]: ht[p, 128*j + b] = h2[b, 128*j + p]
        h2 = ht.reshape(P, KH, P).transpose(2, 1, 0).reshape(P, H)
        outs.append(h2 @ fc_w)
    out = np.concatenate(outs, axis=0)
    return (out + fc_b.reshape(1, -1)).astype(np.float32)
